# revision 1
# baseline (speedup 1.0000x reference)
"""GAT (2-layer graph attention network) on 8 Trainium2 NeuronCores.

Strategy (dst-sharded graph parallel):
  - Nodes are partitioned across the 8 cores (6250 dst nodes each).
  - Host pre-sorts edges (incl. self-loops) by destination, groups them into
    128-dst "windows" and 128-edge "chunks", padding so every core runs an
    identical static program.  Chunks are additionally segregated by source
    half (int16 gather-index range), and batched into 2048-edge superchunks.
  - Per core: h = x_shard @ W1 (+ fused alpha columns), AllGather replicates
    [h | alpha_src] for all nodes.
  - Edge loop: dma_gather fetches source rows, leaky-relu+exp on-chip,
    per-edge weighting (head-interleaved broadcast), one-hot matmul
    scatter-add into PSUM per 128-dst window; softmax denominators from a
    second matmul against the same one-hot.
  - Layer 2 repeats at OUT=40 channels, then log_softmax.
"""

import math
from dataclasses import dataclass

import ml_dtypes
import numpy as np

import concourse.bass as bass
import concourse.mybir as mybir
import concourse.tile as tile
from concourse import bacc

BF16 = ml_dtypes.bfloat16
P = 128
NEG_SLOPE = 0.2
HALF = 25000  # src-table split point (int16 index range)


@dataclass(frozen=True)
class Cfg:
    N: int = 50000
    F: int = 256
    H: int = 8
    C: int = 64
    OUT: int = 40
    NC: int = 8
    SCK: int = 32          # chunks per gather superchunk

    @property
    def HC(self):
        return self.H * self.C

    @property
    def NSH(self):
        return self.N // self.NC

    @property
    def NWIN(self):
        return math.ceil(self.NSH / P)

    @property
    def nclass(self):
        return 2 if self.N > 32767 else 1


FULL = Cfg()


# ---------------------------------------------------------------- host side


def _schedule(cfg: Cfg, edge_index: np.ndarray):
    """Sort/pad edges into the common static chunk schedule.

    Returns (nch, eidx, edl):
      nch  : [nclass][NWIN] chunks per (class, window), common to all cores
      eidx : int16 [NC, TSC, 128, 2, 128]  (plane 0: src local-to-table,
             plane 1: dst local-to-core; idx i of a superchunk at
             [i % 16 (+16g replicas), i // 16])
      edl  : float32 [NC, TSC, 128, SCK] dst-in-window (pad -1)
    """
    N, NCOR, NSH, SCK = cfg.N, cfg.NC, cfg.NSH, cfg.SCK
    NWIN, NCLS = cfg.NWIN, cfg.nclass
    loop = np.arange(N, dtype=np.int64)
    src = np.concatenate([edge_index[0], loop]).astype(np.int64)
    dst = np.concatenate([edge_index[1], loop]).astype(np.int64)
    core = (dst // NSH).astype(np.int32)
    dl = (dst % NSH).astype(np.int32)
    win = dl // P
    wloc = dl % P
    cls = (src >= HALF).astype(np.int32) if NCLS == 2 else np.zeros_like(core)

    counts = np.zeros((NCOR, NCLS, NWIN), np.int64)
    np.add.at(counts, (core, cls, win), 1)
    nch = np.maximum(np.ceil(counts / P).astype(np.int64).max(axis=0), 1)
    T_c = [int(nch[c].sum()) for c in range(NCLS)]
    TSC_c = [math.ceil(t / SCK) for t in T_c]
    TSC = sum(TSC_c)
    TP_c = [t * SCK for t in TSC_c]

    # slot offset of (class, window) inside its class stream (in edges)
    wstart = np.zeros((NCLS, NWIN + 1), np.int64)
    for c in range(NCLS):
        wstart[c, 1:] = np.cumsum(nch[c])
    wstart *= P

    eidx = np.zeros((NCOR, TSC, P, 2, SCK * 8), np.int16)
    edlA = np.full((NCOR, TSC, P, SCK), -1.0, np.float32)
    sc_base = [0, TSC_c[0]] if NCLS == 2 else [0]

    for k in range(NCOR):
        for c in range(NCLS):
            m = (core == k) & (cls == c)
            s_k = src[m] - (HALF if c == 1 else 0)
            w_k = win[m]
            wl_k = wloc[m]
            d_k = dl[m]
            order = np.lexsort((s_k, w_k))
            s_k, w_k, wl_k, d_k = (s_k[order], w_k[order], wl_k[order],
                                   d_k[order])
            cnts = counts[k, c]
            offs = np.concatenate([[0], np.cumsum(cnts[:-1])])
            slot = wstart[c, w_k] + (np.arange(len(s_k)) - offs[w_k])
            es = np.zeros(TP_c[c] * P, np.int16)
            ed = np.zeros(TP_c[c] * P, np.int16)
            el = np.full(TP_c[c] * P, -1.0, np.float32)
            es[slot] = s_k.astype(np.int16)
            ed[slot] = d_k.astype(np.int16)
            el[slot] = wl_k.astype(np.float32)
            # flat i -> idx tile [i%16, i//16] (replicated), edl [i%128, i//128]
            for si in range(TSC_c[c]):
                sc = sc_base[c] + si
                seg_s = es[si * SCK * P:(si + 1) * SCK * P]
                seg_d = ed[si * SCK * P:(si + 1) * SCK * P]
                seg_l = el[si * SCK * P:(si + 1) * SCK * P]
                t16s = seg_s.reshape(-1, 16).T  # [16, SCK*8]
                t16d = seg_d.reshape(-1, 16).T
                eidx[k, sc, :, 0, :] = np.tile(t16s, (8, 1))
                eidx[k, sc, :, 1, :] = np.tile(t16d, (8, 1))
                edlA[k, sc] = seg_l.reshape(SCK, P).T
    nch_py = [[int(x) for x in nch[c]] for c in range(NCLS)]
    return nch_py, TSC_c, eidx, edlA


def _perm(cfg: Cfg):
    p = np.empty(cfg.HC, np.int64)
    for h in range(cfg.H):
        p[np.arange(cfg.C) * cfg.H + h] = h * cfg.C + np.arange(cfg.C)
    return p


def _prep_weights(cfg: Cfg, W1, a_src1, a_dst1, W2, a_src2, a_dst2):
    perm = _perm(cfg)
    H, C, HC, OUT = cfg.H, cfg.C, cfg.HC, cfg.OUT
    Ws1 = np.stack([W1[:, h * C:(h + 1) * C] @ a_src1[h] for h in range(H)], 1)
    Wd1 = np.stack([W1[:, h * C:(h + 1) * C] @ a_dst1[h] for h in range(H)], 1)
    W1aug = np.concatenate([W1[:, perm], Ws1, Wd1], axis=1).astype(BF16)
    w2s = (W2 @ a_src2[0])[:, None]
    w2d = (W2 @ a_dst2[0])[:, None]
    L2PAD = 48 - (OUT + 2)
    W2aug = np.concatenate(
        [W2, w2s, w2d, np.zeros((HC, L2PAD), W2.dtype)], axis=1
    )[perm, :].astype(BF16)
    return W1aug, W2aug


# -------------------------------------------------------------- device side


def _build(cfg: Cfg, nch, TSC_c, b1_nonzero=False, b2_nonzero=False):
    N, F, H, C, HC, OUT = cfg.N, cfg.F, cfg.H, cfg.C, cfg.HC, cfg.OUT
    NSH, SCK, NWIN = cfg.NSH, cfg.SCK, cfg.NWIN
    NCLS = cfg.nclass
    TSC = sum(TSC_c)
    KT = math.ceil(F / P)
    BT = math.ceil(HC / P)
    AUG1 = HC + 2 * H
    AUG2 = 48
    HXW = 640                    # hx table row stride (elements)
    ADW = 128                    # aux table row stride
    NT = NWIN
    XPAD = NT * P
    NIDX = SCK * P

    bf = mybir.dt.bfloat16
    f32 = mybir.dt.float32
    i16 = mybir.dt.int16
    AF = mybir.ActivationFunctionType
    OP = mybir.AluOpType

    nc = bacc.Bacc(
        "TRN2", target_bir_lowering=False, debug=False,
        enable_asserts=False, num_devices=cfg.NC,
    )

    xT_t = nc.dram_tensor("xT", [F, XPAD], bf, kind="ExternalInput")
    w1_t = nc.dram_tensor("W1aug", [F, AUG1], bf, kind="ExternalInput")
    w2_t = nc.dram_tensor("W2aug", [HC, AUG2], bf, kind="ExternalInput")
    iota_t = nc.dram_tensor("iota", [P, P], bf, kind="ExternalInput")
    ident_t = nc.dram_tensor("ident", [P, P], bf, kind="ExternalInput")
    eidx_t = nc.dram_tensor("eidx", [TSC, P, 2, SCK * 8], i16, kind="ExternalInput")
    edl_t = nc.dram_tensor("edl", [TSC, P, SCK], f32, kind="ExternalInput")
    if b1_nonzero:
        b1_t = nc.dram_tensor("b1rep", [P, HC], f32, kind="ExternalInput")
    if b2_nonzero:
        b2_t = nc.dram_tensor("b2rep", [P, OUT], f32, kind="ExternalInput")
    out_t = nc.dram_tensor("out", [NSH, OUT], f32, kind="ExternalOutput")

    # (class, sc, kk) schedule per window: class streams are contiguous
    sc_base = [0, TSC_c[0]] if NCLS == 2 else [0]
    window_chunks = []  # per window: list of (sc, kk)
    pos_c = [0] * NCLS
    for w in range(NWIN):
        lst = []
        for c in range(NCLS):
            for _ in range(nch[c][w]):
                j = pos_c[c]
                lst.append((c, sc_base[c] + j // SCK, j % SCK))
                pos_c[c] += 1
        window_chunks.append(lst)
    rows_of = lambda w: min(P, NSH - w * P)
    # class of each sc (for table selection)
    sc_cls = [0] * TSC
    if NCLS == 2:
        for s in range(TSC_c[0], TSC):
            sc_cls[s] = 1

    with tile.TileContext(nc) as tc:
        dram_pool = tc.tile_pool(name="dram", bufs=1, space="DRAM")
        pdr = dram_pool.__enter__()
        shared_as = "Shared" if cfg.NC > 4 else "Local"
        hx_dram = pdr.tile([NSH, HXW], bf, name="hx_dram")
        adt = pdr.tile([NSH, ADW], bf, name="adt")
        hx_full = pdr.tile([N, HXW], bf, addr_space=shared_as, name="hx_full")
        hx2_dram = pdr.tile([NSH, ADW], bf, name="hx2_dram")
        hx2_full = pdr.tile([N, ADW], bf, addr_space=shared_as,
                            name="hx2_full")

        def src_tab(t, sc, lo_col, hi_col):
            """table slice for superchunk sc's class."""
            if NCLS == 1 or sc_cls[sc] == 0:
                return t[0:min(HALF, N), lo_col:hi_col]
            return t[HALF:N, lo_col:hi_col]

        with tc.tile_pool(name="const", bufs=1) as pc:
            iota_sb = pc.tile([P, P], bf, name="iota_sb")
            nc.sync.dma_start(out=iota_sb[:], in_=iota_t[:, :])
            ident_sb = pc.tile([P, P], bf, name="ident_sb")
            nc.sync.dma_start(out=ident_sb[:], in_=ident_t[:, :])
            w1_sb = []
            for kk in range(KT):
                r = min(P, F - kk * P)
                t_ = pc.tile([r, AUG1], bf, name=f"w1_sb{kk}")
                nc.sync.dma_start(out=t_[:], in_=w1_t[kk * P:kk * P + r, :])
                w1_sb.append(t_)
            w2_sb = []
            for b in range(BT):
                r = min(P, HC - b * P)
                t_ = pc.tile([r, AUG2], bf, name=f"w2_sb{b}")
                nc.sync.dma_start(out=t_[:], in_=w2_t[b * P:b * P + r, :])
                w2_sb.append(t_)
            xT_sb = []
            for kk in range(KT):
                r = min(P, F - kk * P)
                t_ = pc.tile([r, XPAD], bf, name=f"xT_sb{kk}")
                nc.sync.dma_start(out=t_[:], in_=xT_t[kk * P:kk * P + r, :])
                xT_sb.append(t_)
            if b1_nonzero:
                b1_sb = pc.tile([P, HC], f32, name="b1_sb")
                nc.sync.dma_start(out=b1_sb[:], in_=b1_t[:, :])
            if b2_nonzero:
                b2_sb = pc.tile([P, OUT], f32, name="b2_sb")
                nc.sync.dma_start(out=b2_sb[:], in_=b2_t[:, :])

            # ---------------- phase A: [h | a_src | a_dst] = x @ W1aug
            with tc.tile_pool(name="phA_ps", bufs=2, space="PSUM") as pap, \
                 tc.tile_pool(name="phA_sb", bufs=3) as pas:
                for t in range(NT):
                    pn = pap.tile([P, HC], f32, tag="pA", name=f"pA{t}")
                    pa = pap.tile([P, 2 * H], f32, tag="pB", name=f"pB{t}")
                    for kk in range(KT):
                        lhsT = xT_sb[kk][:, t * P:(t + 1) * P]
                        nc.tensor.matmul(
                            out=pn[:], lhsT=lhsT, rhs=w1_sb[kk][:, 0:HC],
                            start=(kk == 0), stop=(kk == KT - 1))
                        nc.tensor.matmul(
                            out=pa[:], lhsT=lhsT, rhs=w1_sb[kk][:, HC:AUG1],
                            start=(kk == 0), stop=(kk == KT - 1))
                    hxa = pas.tile([P, HC + 2 * H], bf, tag="hxa",
                                   name=f"hxa{t}")
                    nc.scalar.activation(hxa[:, 0:HC], pn[:], AF.Copy)
                    nc.scalar.activation(hxa[:, HC:HC + 2 * H], pa[:], AF.Copy)
                    ada = pas.tile([P, H], bf, tag="ada", name=f"ada{t}")
                    nc.vector.tensor_copy(out=ada[:], in_=pa[:, H:2 * H])
                    r = rows_of(t)
                    nc.sync.dma_start(
                        out=hx_dram[t * P:t * P + r, 0:HC + 2 * H],
                        in_=hxa[:r, :])
                    nc.sync.dma_start(
                        out=adt[t * P:t * P + r, 0:H], in_=ada[:r, :])

            # ---------------- phase B: AllGather [h | alpha_src | ...]
            nc.gpsimd.collective_compute(
                "AllGather", OP.bypass,
                replica_groups=[list(range(cfg.NC))],
                ins=[hx_dram.opt()], outs=[hx_full.opt()],
            )

            # ---------------- phase C: layer-1 edge loop (+ h2 per window)
            with tc.tile_pool(name="phC_st", bufs=2) as pst, \
                 tc.tile_pool(name="phC_ck", bufs=4) as pck, \
                 tc.tile_pool(name="phC_po", bufs=2) as ppo, \
                 tc.tile_pool(name="phC_ps", bufs=2, space="PSUM") as pps, \
                 tc.tile_pool(name="phC_pt", bufs=1, space="PSUM") as ppt:

                sc_cache = {}

                def get_sc(sc):
                    if sc in sc_cache:
                        return sc_cache[sc]
                    six = pst.tile([P, SCK * 8], i16, tag="six", name=f"six{sc}")
                    nc.sync.dma_start(out=six[:], in_=eidx_t[sc, :, 0, :])
                    dix = pst.tile([P, SCK * 8], i16, tag="dix", name=f"dix{sc}")
                    nc.sync.dma_start(out=dix[:], in_=eidx_t[sc, :, 1, :])
                    edl = pst.tile([P, SCK], f32, tag="edl", name=f"edl{sc}")
                    nc.sync.dma_start(out=edl[:], in_=edl_t[sc, :, :])
                    hxg = pst.tile([P, SCK, HC], bf, tag="hxg",
                                   name=f"hxg{sc}")
                    nc.gpsimd.dma_gather(
                        hxg[:], src_tab(hx_full, sc, 0, HC), six[:],
                        NIDX, NIDX, HC, elem_step=HXW, single_packet=False)
                    asg = pst.tile([P, SCK, ADW], bf, tag="asg",
                                   name=f"asg{sc}")
                    nc.gpsimd.dma_gather(
                        asg[:], src_tab(hx_full, sc, HC, HXW), six[:],
                        NIDX, NIDX, ADW, elem_step=HXW, single_packet=False)
                    adg = pst.tile([P, SCK, ADW], bf, tag="adg",
                                   name=f"adg{sc}")
                    nc.gpsimd.dma_gather(
                        adg[:], adt[:, :], dix[:],
                        NIDX, NIDX, ADW, single_packet=False)
                    es = pst.tile([P, SCK, H], f32, tag="es", name=f"es{sc}")
                    nc.vector.tensor_tensor(
                        out=es[:], in0=asg[:, :, 0:H], in1=adg[:, :, 0:H],
                        op=OP.add)
                    elr = pst.tile([P, SCK, H], f32, tag="elr",
                                   name=f"elr{sc}")
                    nc.vector.scalar_tensor_tensor(
                        out=elr[:], in0=es[:], scalar=NEG_SLOPE, in1=es[:],
                        op0=OP.mult, op1=OP.max)
                    wts = pst.tile([P, SCK, H], bf, tag="wts", name=f"wts{sc}")
                    nc.scalar.activation(wts[:], elr[:], AF.Exp)
                    sc_cache[sc] = (hxg, wts, edl)
                    return sc_cache[sc]

                for w in range(NWIN):
                    pn1 = pps.tile([P, HC], f32, tag="pn1", name=f"pn1_{w}")
                    pd1 = pps.tile([P, H], f32, tag="pd1", name=f"pd1_{w}")
                    njw = len(window_chunks[w])
                    for i, (c, sc, kk) in enumerate(window_chunks[w]):
                        hxg, wts, edl = get_sc(sc)
                        oh = pck.tile([P, P], bf, tag="oh", name=f"oh{w}_{i}")
                        nc.vector.tensor_scalar(
                            out=oh[:], in0=iota_sb[:],
                            scalar1=edl[:, kk:kk + 1], scalar2=None,
                            op0=OP.is_equal)
                        msg = pck.tile([P, HC], bf, tag="msg",
                                       name=f"msg{w}_{i}")
                        nc.vector.tensor_tensor(
                            out=msg[:].rearrange("p (c h) -> p c h", h=H),
                            in0=hxg[:, kk, :].rearrange(
                                "p (c h) -> p c h", h=H),
                            in1=wts[:, kk:kk + 1, :].to_broadcast([P, C, H]),
                            op=OP.mult)
                        nc.tensor.matmul(
                            out=pn1[:], lhsT=oh[:], rhs=msg[:],
                            start=(i == 0), stop=(i == njw - 1))
                        nc.tensor.matmul(
                            out=pd1[:], lhsT=oh[:], rhs=wts[:, kk, :],
                            start=(i == 0), stop=(i == njw - 1))

                    den = ppo.tile([P, H], f32, tag="den", name=f"den{w}")
                    nc.vector.tensor_scalar(
                        out=den[:], in0=pd1[:], scalar1=1e-30, scalar2=None,
                        op0=OP.add)
                    rden = ppo.tile([P, H], f32, tag="rden", name=f"rden{w}")
                    nc.vector.reciprocal(out=rden[:], in_=den[:])
                    h1a = ppo.tile([P, HC], bf, tag="h1a", name=f"h1a{w}")
                    h1v = h1a[:].rearrange("p (c h) -> p c h", h=H)
                    pnv = pn1[:].rearrange("p (c h) -> p c h", h=H)
                    if not b1_nonzero:
                        for h in range(H):
                            nc.scalar.activation(
                                h1v[:, :, h:h + 1], pnv[:, :, h:h + 1],
                                AF.Relu, scale=rden[:, h:h + 1])
                    else:
                        t1 = ppo.tile([P, HC], f32, tag="t1", name=f"t1_{w}")
                        t1v = t1[:].rearrange("p (c h) -> p c h", h=H)
                        for h in range(H):
                            nc.scalar.activation(
                                t1v[:, :, h:h + 1], pnv[:, :, h:h + 1],
                                AF.Copy, scale=rden[:, h:h + 1])
                        nc.vector.tensor_tensor(
                            out=t1[:], in0=t1[:], in1=b1_sb[:], op=OP.add)
                        nc.vector.tensor_scalar(
                            out=h1a[:], in0=t1[:], scalar1=0.0, scalar2=None,
                            op0=OP.max)

                    # layer-2 pre-pass for this node tile
                    ph2 = pps.tile([P, AUG2], f32, tag="ph2", name=f"ph2_{w}")
                    for b in range(BT):
                        r = min(P, HC - b * P)
                        tp = ppt.tile([P, P], bf, tag="tp", name=f"tp{w}_{b}")
                        nc.tensor.transpose(
                            out=tp[:r, :], in_=h1a[:, b * P:b * P + r],
                            identity=ident_sb[:])
                        h1T = ppo.tile([P, P], bf, tag="h1T",
                                       name=f"h1T{w}_{b}")
                        nc.scalar.activation(h1T[:r, :], tp[:r, :], AF.Copy)
                        nc.tensor.matmul(
                            out=ph2[:], lhsT=h1T[:r, :], rhs=w2_sb[b][:],
                            start=(b == 0), stop=(b == BT - 1))
                    hx2a = ppo.tile([P, AUG2], bf, tag="hx2a", name=f"hx2a{w}")
                    nc.scalar.activation(hx2a[:], ph2[:], AF.Copy)
                    ad2a = ppo.tile([P, 1], bf, tag="ad2a", name=f"ad2a{w}")
                    nc.vector.tensor_copy(
                        out=ad2a[:], in_=ph2[:, OUT + 1:OUT + 2])
                    r = rows_of(w)
                    nc.sync.dma_start(
                        out=hx2_dram[w * P:w * P + r, 0:AUG2], in_=hx2a[:r, :])
                    nc.sync.dma_start(
                        out=adt[w * P:w * P + r, H:H + 1], in_=ad2a[:r, :])

            # ---------------- AllGather layer-2 features
            nc.gpsimd.collective_compute(
                "AllGather", OP.bypass,
                replica_groups=[list(range(cfg.NC))],
                ins=[hx2_dram.opt()], outs=[hx2_full.opt()],
            )

            # ---------------- phase D: layer-2 edge loop + log_softmax
            with tc.tile_pool(name="phD_st", bufs=2) as pst, \
                 tc.tile_pool(name="phD_ck", bufs=4) as pck, \
                 tc.tile_pool(name="phD_po", bufs=2) as ppo, \
                 tc.tile_pool(name="phD_ps", bufs=2, space="PSUM") as pps:

                sc2_cache = {}

                def get_sc2(sc):
                    if sc in sc2_cache:
                        return sc2_cache[sc]
                    six = pst.tile([P, SCK * 8], i16, tag="six2", name=f"s2ix{sc}")
                    nc.sync.dma_start(out=six[:], in_=eidx_t[sc, :, 0, :])
                    dix = pst.tile([P, SCK * 8], i16, tag="dix2", name=f"d2ix{sc}")
                    nc.sync.dma_start(out=dix[:], in_=eidx_t[sc, :, 1, :])
                    edl = pst.tile([P, SCK], f32, tag="edl2",
                                   name=f"edl2_{sc}")
                    nc.sync.dma_start(out=edl[:], in_=edl_t[sc, :, :])
                    hxg = pst.tile([P, SCK, ADW], bf, tag="hxg2",
                                   name=f"hxg2_{sc}")
                    nc.gpsimd.dma_gather(
                        hxg[:], src_tab(hx2_full, sc, 0, ADW), six[:],
                        NIDX, NIDX, ADW, single_packet=False)
                    adg = pst.tile([P, SCK, ADW], bf, tag="adg2",
                                   name=f"adg2_{sc}")
                    nc.gpsimd.dma_gather(
                        adg[:], adt[:, :], dix[:],
                        NIDX, NIDX, ADW, single_packet=False)
                    es = pst.tile([P, SCK, 1], f32, tag="es2",
                                  name=f"es2_{sc}")
                    nc.vector.tensor_tensor(
                        out=es[:], in0=hxg[:, :, OUT:OUT + 1],
                        in1=adg[:, :, H:H + 1], op=OP.add)
                    elr = pst.tile([P, SCK, 1], f32, tag="elr2",
                                   name=f"elr2_{sc}")
                    nc.vector.scalar_tensor_tensor(
                        out=elr[:], in0=es[:], scalar=NEG_SLOPE, in1=es[:],
                        op0=OP.mult, op1=OP.max)
                    wts = pst.tile([P, SCK, 1], f32, tag="wts2",
                                   name=f"wts2_{sc}")
                    nc.scalar.activation(wts[:], elr[:], AF.Exp)
                    wtsb = pst.tile([P, SCK, 1], bf, tag="wtsb2",
                                    name=f"wtsb2_{sc}")
                    nc.vector.tensor_copy(out=wtsb[:], in_=wts[:])
                    sc2_cache[sc] = (hxg, wts, wtsb, edl)
                    return sc2_cache[sc]

                for w in range(NWIN):
                    pn2 = pps.tile([P, OUT], f32, tag="pn2", name=f"pn2_{w}")
                    pd2 = pps.tile([P, 1], f32, tag="pd2", name=f"pd2_{w}")
                    njw = len(window_chunks[w])
                    for i, (c, sc, kk) in enumerate(window_chunks[w]):
                        hxg, wts, wtsb, edl = get_sc2(sc)
                        oh = pck.tile([P, P], bf, tag="oh2",
                                      name=f"o2h{w}_{i}")
                        nc.vector.tensor_scalar(
                            out=oh[:], in0=iota_sb[:],
                            scalar1=edl[:, kk:kk + 1], scalar2=None,
                            op0=OP.is_equal)
                        msg = pck.tile([P, OUT], bf, tag="msg2",
                                       name=f"ms2{w}_{i}")
                        nc.vector.tensor_scalar(
                            out=msg[:], in0=hxg[:, kk, 0:OUT],
                            scalar1=wts[:, kk, :], scalar2=None, op0=OP.mult)
                        nc.tensor.matmul(
                            out=pn2[:], lhsT=oh[:], rhs=msg[:],
                            start=(i == 0), stop=(i == njw - 1))
                        nc.tensor.matmul(
                            out=pd2[:], lhsT=oh[:], rhs=wtsb[:, kk, :],
                            start=(i == 0), stop=(i == njw - 1))

                    den = ppo.tile([P, 1], f32, tag="den2", name=f"den2_{w}")
                    nc.vector.tensor_scalar(
                        out=den[:], in0=pd2[:], scalar1=1e-30, scalar2=None,
                        op0=OP.add)
                    rden = ppo.tile([P, 1], f32, tag="rden2", name=f"rd2_{w}")
                    nc.vector.reciprocal(out=rden[:], in_=den[:])
                    o2 = ppo.tile([P, OUT], f32, tag="o2", name=f"o2_{w}")
                    nc.scalar.activation(
                        o2[:], pn2[:, 0:OUT], AF.Copy, scale=rden[:, 0:1])
                    if b2_nonzero:
                        nc.vector.tensor_tensor(
                            out=o2[:], in0=o2[:], in1=b2_sb[:], op=OP.add)
                    mx = ppo.tile([P, 1], f32, tag="mx", name=f"mx{w}")
                    nc.vector.reduce_max(
                        out=mx[:], in_=o2[:], axis=mybir.AxisListType.X)
                    negm = ppo.tile([P, 1], f32, tag="negm", name=f"negm{w}")
                    nc.vector.tensor_scalar(
                        out=negm[:], in0=mx[:], scalar1=-1.0, scalar2=None,
                        op0=OP.mult)
                    ex = ppo.tile([P, OUT], f32, tag="ex", name=f"ex{w}")
                    ssum = ppo.tile([P, 1], f32, tag="ssum", name=f"ssum{w}")
                    nc.scalar.activation(
                        ex[:], o2[:], AF.Exp, bias=negm[:, 0:1],
                        accum_out=ssum[:, 0:1])
                    lns = ppo.tile([P, 1], f32, tag="lns", name=f"lns{w}")
                    nc.scalar.activation(lns[:], ssum[:], AF.Ln)
                    sh = ppo.tile([P, 1], f32, tag="sh", name=f"sh{w}")
                    nc.vector.tensor_tensor(
                        out=sh[:], in0=negm[:], in1=lns[:], op=OP.subtract)
                    outt = ppo.tile([P, OUT], f32, tag="outt", name=f"outt{w}")
                    nc.scalar.activation(
                        outt[:], o2[:], AF.Identity, bias=sh[:, 0:1])
                    r = rows_of(w)
                    nc.sync.dma_start(
                        out=out_t[w * P:w * P + r, :], in_=outt[:r, :])

        dram_pool.__exit__(None, None, None)

    nc.compile()
    return nc


# ------------------------------------------------------------------ driver


def make_in_maps(cfg: Cfg, inputs: dict):
    x = np.asarray(inputs["x"], np.float32)
    edge_index = np.asarray(inputs["edge_index"])
    W1 = np.asarray(inputs["W1"], np.float32)
    a_src1 = np.asarray(inputs["a_src1"], np.float32)
    a_dst1 = np.asarray(inputs["a_dst1"], np.float32)
    b1 = np.asarray(inputs["b1"], np.float32)
    W2 = np.asarray(inputs["W2"], np.float32)
    a_src2 = np.asarray(inputs["a_src2"], np.float32)
    a_dst2 = np.asarray(inputs["a_dst2"], np.float32)
    b2 = np.asarray(inputs["b2"], np.float32)

    nch, TSC_c, eidx, edl = _schedule(cfg, edge_index)
    W1aug, W2aug = _prep_weights(cfg, W1, a_src1, a_dst1, W2, a_src2, a_dst2)
    iota = np.tile(np.arange(P, dtype=BF16), (P, 1))
    ident = np.eye(P, dtype=BF16)
    b1_nonzero = bool(np.any(b1))
    b2_nonzero = bool(np.any(b2))
    perm = _perm(cfg)

    NT = cfg.NWIN
    XPAD = NT * P
    in_maps = []
    for k in range(cfg.NC):
        xs = x[k * cfg.NSH:(k + 1) * cfg.NSH]
        xTp = np.zeros((cfg.F, XPAD), BF16)
        xTp[:, :cfg.NSH] = xs.T.astype(BF16)
        m = {
            "xT": xTp,
            "W1aug": W1aug,
            "W2aug": W2aug,
            "iota": iota,
            "ident": ident,
            "eidx": eidx[k],
            "edl": edl[k],
        }
        if b1_nonzero:
            m["b1rep"] = np.tile(b1[perm][None, :], (P, 1)).astype(np.float32)
        if b2_nonzero:
            m["b2rep"] = np.tile(b2[None, :], (P, 1)).astype(np.float32)
        in_maps.append(m)
    return in_maps, nch, TSC_c, b1_nonzero, b2_nonzero


class Executor:
    """Compile once; execute repeatedly through one jitted shard_map."""

    def __init__(self, cfg: Cfg, nch, TSC_c, b1nz, b2nz):
        import jax
        from jax.sharding import Mesh, PartitionSpec
        from jax.experimental.shard_map import shard_map
        from concourse import bass2jax
        import concourse.mybir as mybir_

        self.cfg = cfg
        nc = _build(cfg, nch, TSC_c, b1nz, b2nz)
        self.nc = nc
        bass2jax.install_neuronx_cc_hook()

        in_names, out_names, out_avals, zero_shapes = [], [], [], []
        for alloc in nc.m.functions[0].allocations:
            if not isinstance(alloc, mybir_.MemoryLocationSet):
                continue
            name = alloc.memorylocations[0].name
            if alloc.kind == "ExternalInput":
                in_names.append(name)
            elif alloc.kind == "ExternalOutput":
                shape = tuple(alloc.tensor_shape)
                dtype = mybir_.dt.np(alloc.dtype)
                out_avals.append(jax.core.ShapedArray(shape, dtype))
                out_names.append(name)
                zero_shapes.append((shape, dtype))
        assert nc.dbg_addr is None
        part_name = (nc.partition_id_tensor.name
                     if nc.partition_id_tensor else None)
        in_names = [n for n in in_names if n != part_name]
        n_params = len(in_names)
        all_names = in_names + out_names
        if part_name is not None:
            all_names = all_names + [part_name]
        donate = tuple(range(n_params, n_params + len(out_names)))

        def _body(*args):
            operands = list(args)
            if part_name is not None:
                operands.append(bass2jax.partition_id_tensor())
            outs = bass2jax._bass_exec_p.bind(
                *operands,
                out_avals=tuple(out_avals),
                in_names=tuple(all_names),
                out_names=tuple(out_names),
                lowering_input_output_aliases=(),
                sim_require_finite=True,
                sim_require_nnan=True,
                nc=nc,
            )
            return tuple(outs)

        devices = jax.devices()[:cfg.NC]
        mesh = Mesh(np.asarray(devices), ("core",))
        nio = n_params + len(out_names)
        self._fn = jax.jit(
            shard_map(
                _body, mesh=mesh,
                in_specs=(PartitionSpec("core"),) * nio,
                out_specs=(PartitionSpec("core"),) * len(out_names),
                check_rep=False,
            ),
            donate_argnums=donate, keep_unused=True,
        )
        self.in_names = in_names
        self.out_names = out_names
        self.out_avals = out_avals
        self.zero_shapes = zero_shapes
        self.mesh = mesh

    def bench(self, in_maps, iters=10):
        """Device-resident repeat timing: inputs uploaded once, outputs
        chained through donation. Returns per-iteration wall seconds."""
        import time
        import jax
        from jax.sharding import NamedSharding, PartitionSpec
        cfg = self.cfg
        sh = NamedSharding(self.mesh, PartitionSpec("core"))
        din = [
            jax.device_put(
                np.concatenate([np.asarray(m[n]) for m in in_maps], axis=0),
                sh)
            for n in self.in_names
        ]
        prev = [
            jax.device_put(np.zeros((cfg.NC * s[0], *s[1:]), d), sh)
            for s, d in self.zero_shapes
        ]
        outs = self._fn(*din, *prev)  # warm
        jax.block_until_ready(outs)
        times = []
        for _ in range(iters):
            t0 = time.perf_counter()
            outs = self._fn(*din, *outs)
            jax.block_until_ready(outs)
            times.append(time.perf_counter() - t0)
        return times

    def __call__(self, in_maps):
        cfg = self.cfg
        concat_in = [
            np.concatenate([np.asarray(m[n]) for m in in_maps], axis=0)
            for n in self.in_names
        ]
        concat_zeros = [
            np.zeros((cfg.NC * s[0], *s[1:]), d) for s, d in self.zero_shapes
        ]
        outs = self._fn(*concat_in, *concat_zeros)
        outs = [np.asarray(o) for o in outs]
        return [
            {
                n: outs[i].reshape(cfg.NC, *self.out_avals[i].shape)[c]
                for i, n in enumerate(self.out_names)
            }
            for c in range(cfg.NC)
        ]


_exec_cache = {}


def get_executor(cfg: Cfg, inputs: dict):
    in_maps, nch, TSC_c, b1nz, b2nz = make_in_maps(cfg, inputs)
    key = (cfg, tuple(tuple(c) for c in nch), b1nz, b2nz)
    if key not in _exec_cache:
        _exec_cache[key] = Executor(cfg, nch, TSC_c, b1nz, b2nz)
    return _exec_cache[key], in_maps


def run(cfg: Cfg, inputs: dict, trace: bool = False):
    ex, in_maps = get_executor(cfg, inputs)
    results = ex(in_maps)
    out = np.concatenate([results[k]["out"] for k in range(cfg.NC)], 0)
    return out, ex


def kernel(**inputs) -> np.ndarray:
    out, _ = run(FULL, inputs)
    return out.astype(np.float32)

